# revision 25
# baseline (speedup 1.0000x reference)
"""EdgeConv message-passing kernel for 8 Trainium2 NeuronCores.

Strategy (host-materialized edge tensor + dense streaming device kernel):
  - Queries are range-partitioned into 4 groups; refs into 2 halves. Core c
    handles query group c>>1 and ref half c&1 (its edges are those with
    e_query in the group and e_ref in the half).
  - All BatchNorms fold at inference:
        pre-relu edge feature  z_e = Z[e_ref] - B[e_query]
        Z[n] = ref_bxyz[n,1:4] @ Wp' + ref_feat[n] @ Wf' + (t0+tf)  [N_ref,32]
        B[q] = q_bxyz[q,1:4] @ Wp'                                  [N_q, 32]
        h_e  = relu(z_e) @ W1' + b1',   out[q] = relu(max_e h_e)  (empty -> 0)
    Using relu(Z-B) = max(Z,B) - B and max_e (x_e - c_q) = (max_e x_e) - c_q:
        out[q] = relu( max_e (max(Z[e_ref],B[q]) @ W1') - C'[q] ),
        C'[q] = B[q] @ W1' - b1'.
  - The host materializes the gather Z[e_ref] per edge slot (pure input
    permutation, like the baseline's packing) in fp16, already laid out as
    the device wants it: queries degree-sorted into tiles of 128; slots
    padded to a multiple of 4 (repeating real edges - idempotent under max);
    groups of 4 slot-blocks stacked on the 128 SBUF partitions (panels).
  - The device streams the edge tensor with dense HWDGE DMA and runs, per
    128-query tile: a DVE max against the resident B table (fp16 2x mode),
    one block-diagonal W1' matmul per <=4 panels (fp16), an ACT-engine
    PSUM -> fp16 SBUF copy, and a DVE tensor-tensor max tree over panels.
    Output [128, qg_pad] fp16 is DMA'd out in chunks as tiles finish.
  - Host post: max over the 4 partition blocks, relu(red - C'), empty-query
    zeroing, inverse permutation, and pairwise max of the two ref halves.
"""
import time

import numpy as np

import concourse.bass as bass
import concourse.tile as tile
from concourse import bacc, mybir

EPS = 1e-3
P = 128
DMA_COLS = 8192      # max zg columns per streamed DMA group
PASS_BLOCKS = 4      # slot-blocks stacked per panel (partition blocks)
PASS_PANELS = 4      # panels per matmul pass (<= 512 psum cols)
DIRECT_EVERY = 10**9  # disabled: direct pair-reduce cost DVE more than it saved

TIMES = {}


def _fold_weights(inputs):
    f = np.float32
    s0 = inputs["bn0_g"] / np.sqrt(inputs["bn0_v"] + EPS)
    t0 = inputs["bn0_b"] - inputs["bn0_m"] * s0
    sf = inputs["bnf_g"] / np.sqrt(inputs["bnf_v"] + EPS)
    tf = inputs["bnf_b"] - inputs["bnf_m"] * sf
    s1 = inputs["bn1_g"] / np.sqrt(inputs["bn1_v"] + EPS)
    t1 = inputs["bn1_b"] - inputs["bn1_m"] * s1

    Wp = (np.asarray(inputs["w_pos"]) * s0).astype(f)      # [3, 32]
    Wf = (np.asarray(inputs["w_feat"]) * sf).astype(f)     # [16, 32]
    cz = (t0 + tf).astype(f)                               # [32]
    W1 = (np.asarray(inputs["w1"]) * s1).astype(np.float16)
    c1 = (np.asarray(inputs["b1"]) * s1 + t1).astype(f)    # [32]

    w1bd = np.zeros((P, P), np.float16)
    for b in range(4):
        w1bd[b * 32:(b + 1) * 32, b * 32:(b + 1) * 32] = W1

    Z = (np.asarray(inputs["ref_bxyz"])[:, 1:4] @ Wp
         + np.asarray(inputs["ref_feat"]) @ Wf + cz).astype(np.float16)
    return {"Wp": Wp, "W1": W1, "c1": c1, "w1bd": w1bd, "Z": Z}


def _plan(inputs):
    """Host-side partitioning: per-core tile schedules (int bookkeeping)."""
    e_ref = np.asarray(inputs["e_ref"]).astype(np.int64)
    e_query = np.asarray(inputs["e_query"]).astype(np.int64)
    n_ref = inputs["ref_bxyz"].shape[0]
    n_q = inputs["query_bxyz"].shape[0]
    half = (n_ref + 1) // 2
    qg = (n_q + 3) // 4
    qg_pad = ((qg + P - 1) // P) * P
    n_tiles = qg_pad // P
    n_dummy = qg_pad - qg

    cores = []
    for g in range(4):
        qlo, qhi = g * qg, min((g + 1) * qg, n_q)
        for h in range(2):
            m = (e_query >= qlo) & (e_query < qhi) & \
                (e_ref >= h * half) & (e_ref < min((h + 1) * half, n_ref))
            er = (e_ref[m] - h * half).astype(np.int64)
            eq = (e_query[m] - qlo).astype(np.int64)
            deg = np.bincount(eq, minlength=qg)
            order = np.argsort(eq, kind="stable")
            er_s = er[order]
            ptr = np.zeros(qg + 1, np.int64)
            np.cumsum(deg, out=ptr[1:])
            perm = np.argsort(deg, kind="stable")      # ascending degree
            qrow = np.full(qg_pad, -1, np.int64)
            qrow[n_dummy:] = perm
            degrow = np.zeros(qg_pad, np.int64)
            degrow[n_dummy:] = deg[perm]
            ptrrow = np.zeros(qg_pad, np.int64)
            ptrrow[n_dummy:] = ptr[perm]
            kt = degrow.reshape(n_tiles, P).max(axis=1)
            kt = np.maximum(kt, 1)
            cores.append({
                "g": g, "h": h, "qlo": qlo, "nq_local": qhi - qlo,
                "er_s": er_s, "deg": deg, "qrow": qrow,
                "degrow": degrow, "ptrrow": ptrrow, "kt": kt,
            })

    # shared slot schedule across the 8 SPMD cores; blocks padded to mult of 4
    kmax = np.max(np.stack([c["kt"] for c in cores]), axis=0)
    k4 = ((kmax + PASS_BLOCKS - 1) // PASS_BLOCKS) * PASS_BLOCKS
    panels = k4 // PASS_BLOCKS                     # panels per tile
    cols = panels * P                              # zg columns per tile
    col_off = np.zeros(n_tiles + 1, np.int64)
    np.cumsum(cols, out=col_off[1:])

    # dma groups: consecutive tiles, <= DMA_COLS columns each
    groups = []
    t = 0
    while t < n_tiles:
        t0_, n = t, 0
        while t < n_tiles and n + int(cols[t]) <= DMA_COLS:
            n += int(cols[t])
            t += 1
        assert t > t0_, f"tile {t0_} alone exceeds DMA_COLS"
        groups.append((t0_, t, n))

    meta = {
        "half": half, "qg": qg, "qg_pad": qg_pad, "n_tiles": n_tiles,
        "n_dummy": n_dummy, "panels": panels, "col_off": col_off,
        "groups": groups, "n_q": n_q, "n_ref": n_ref,
        "totcol": int(col_off[-1]),
    }
    return cores, meta


def _build_core_arrays(core, meta, inputs, folded):
    """zg [128, totcol] f16, b4 [128, qg_pad] f16, Cq [qg_pad, 32] f32."""
    half, qg_pad, n_tiles = meta["half"], meta["qg_pad"], meta["n_tiles"]
    panels, col_off = meta["panels"], meta["col_off"]
    n_ref = meta["n_ref"]
    er_s, degrow, ptrrow, qrow = (core["er_s"], core["degrow"],
                                  core["ptrrow"], core["qrow"])

    lo = core["h"] * half
    hi = min(lo + half, n_ref)
    Zl = np.zeros((hi - lo + 1, 32), np.float16)   # +1: safe row for deg 0
    Zl[:hi - lo] = folded["Z"][lo:hi]

    # flat [4, totcol] table-row index: tile t, panel j, block i=partition
    # block, query p  ->  column col_off[t] + j*128 + p, row idx of slot
    # (t, c=4j+i, p)
    idx4 = np.empty((4, meta["totcol"]), np.int64)
    for t in range(n_tiles):
        rows = slice(t * P, (t + 1) * P)
        K4 = int(panels[t]) * PASS_BLOCKS
        d = np.maximum(degrow[rows], 1)[:, None]
        j = np.arange(K4)[None, :]
        pos = ptrrow[rows][:, None] + (j % d)          # [128, K4]
        if er_s.size:
            it = er_s[np.minimum(pos, er_s.size - 1)]
        else:
            it = np.zeros((P, K4), np.int64)
        it = np.where(degrow[rows][:, None] > 0, it, hi - lo)
        # [128 p, K4] -> [G, 4, 128] -> [4, G*128]
        itt = it.T.reshape(K4 // 4, 4, P).transpose(1, 0, 2).reshape(4, -1)
        idx4[:, col_off[t]:col_off[t + 1]] = itt
    # one big gather + partition-major layout
    zg = np.ascontiguousarray(
        Zl[idx4].transpose(0, 2, 1).reshape(P, meta["totcol"]))

    qx = np.zeros((qg_pad, 3), np.float32)
    valid = qrow >= 0
    qx[valid] = np.asarray(inputs["query_bxyz"])[core["qlo"] + qrow[valid], 1:4]
    B = (qx @ folded["Wp"]).astype(np.float16)         # [qg_pad, 32]
    b4 = np.ascontiguousarray(np.tile(B.T, (4, 1)))    # [128, qg_pad] f16
    Cq = (B.astype(np.float32) @ folded["W1"].astype(np.float32)
          - folded["c1"]).astype(np.float32)           # [qg_pad, 32]
    return zg, b4, Cq


def _build_program(meta):
    f16 = mybir.dt.float16
    f32 = mybir.dt.float32
    qg_pad, n_tiles = meta["qg_pad"], meta["n_tiles"]
    panels, col_off, groups = meta["panels"], meta["col_off"], meta["groups"]

    nc = bacc.Bacc("TRN2", num_devices=8)
    zg_d = nc.dram_tensor("zg", [P, meta["totcol"]], f16, kind="ExternalInput")
    b4_d = nc.dram_tensor("b4", [P, qg_pad], f16, kind="ExternalInput")
    w1_d = nc.dram_tensor("w1bd", [P, P], f16, kind="ExternalInput")
    out_d = nc.dram_tensor("out", [P, qg_pad], f16, kind="ExternalOutput")

    with tile.TileContext(nc) as tc:
        with tc.tile_pool(name="const", bufs=1) as cp, \
             tc.tile_pool(name="bpool", bufs=3) as bp, \
             tc.tile_pool(name="zpool", bufs=3) as zp, \
             tc.tile_pool(name="epool", bufs=4) as ep, \
             tc.tile_pool(name="hpool", bufs=6) as hp, \
             tc.tile_pool(name="ppool", bufs=8) as pp, \
             tc.tile_pool(name="hps", bufs=2, space="PSUM") as hps, \
             tc.tile_pool(name="hps2", bufs=3, space="PSUM") as hps2:
            w1_t = cp.tile([P, P], f16, name="w1_t")
            nc.sync.dma_start(out=w1_t[:], in_=w1_d[:])
            out_stage = cp.tile([P, qg_pad], f16, name="out_stage")

            def tt_max(dst, a, b):
                nc.vector.tensor_tensor(out=dst, in0=a, in1=b,
                                        op=mybir.AluOpType.max)

            HMAX = 10 * P        # per-tile h16 staging width (max panels)

            # pending G=4 tiles whose ttA halves sit in the current W tile
            wstate = {"tile": None, "n": 0, "t0": -1}
            pair_ctr = [0]

            def flush_w():
                n, t0 = wstate["n"], wstate["t0"]
                if wstate["tile"] is None or n == 0:
                    return
                wt = wstate["tile"]
                dst = out_stage[:, t0 * P:(t0 + n) * P]
                if n == 1:
                    tt_max(dst, wt[:, 0:P], wt[:, P:2 * P])
                else:
                    tt_max(dst,
                           wt[:].rearrange("p (k h) -> p k h", h=2 * P)
                               [:, :n, 0:P],
                           wt[:].rearrange("p (k h) -> p k h", h=2 * P)
                               [:, :n, P:2 * P])
                wstate["tile"] = None
                wstate["n"] = 0

            def w_add(t, h16, off):
                """ttA for a G=4 tile's 4 panels (at h16 col off) into W."""
                if wstate["tile"] is not None and \
                        wstate["t0"] + wstate["n"] != t:
                    flush_w()
                if wstate["tile"] is None:
                    wstate["tile"] = pp.tile([P, 4 * 2 * P], f16,
                                             tag="W", name="w_t")
                    wstate["t0"] = t
                    wstate["n"] = 0
                k = wstate["n"]
                tt_max(wstate["tile"][:, k * 2 * P:(k + 1) * 2 * P],
                       h16[:, off:off + 2 * P],
                       h16[:, off + 2 * P:off + 4 * P])
                wstate["n"] += 1
                if wstate["n"] == 4:
                    flush_w()

            for (ta, tb, ncols) in groups:
                zg_t = zp.tile([P, DMA_COLS], f16, tag="zg")
                base = int(col_off[ta])
                nc.sync.dma_start(out=zg_t[:, :ncols],
                                  in_=zg_d[:, base:base + ncols])
                b4_t = bp.tile([P, DMA_COLS], f16, tag="b4")
                nc.sync.dma_start(out=b4_t[:, :(tb - ta) * P],
                                  in_=b4_d[:, ta * P:tb * P])
                t = ta
                while t < tb:
                    G = int(panels[t])
                    toff = int(col_off[t]) - base
                    b4_s = b4_t[:, (t - ta) * P:(t - ta + 1) * P]
                    ostage = out_stage[:, t * P:(t + 1) * P]

                    if G == 4 and t + 1 < tb and int(panels[t + 1]) == 4:
                        # pair of G=4 tiles: one wide m-max, 2-bank psum,
                        # one eviction (or one paired direct reduce)
                        pair_ctr[0] += 1
                        W2 = 8 * P
                        ecat = ep.tile([P, W2], f16, tag="e2",
                                       name="ecat2")
                        nc.vector.tensor_tensor(
                            out=ecat[:].rearrange("p (u j q) -> p u j q",
                                                  u=2, q=P),
                            in0=zg_t[:, toff:toff + W2]
                                .rearrange("p (u j q) -> p u j q", u=2, q=P),
                            in1=b4_t[:, (t - ta) * P:(t - ta + 2) * P]
                                .rearrange("p (u j q) -> p u j q", u=2, j=1)
                                .to_broadcast([P, 2, PASS_PANELS, P]),
                            op=mybir.AluOpType.max)
                        psum2 = hps2.tile([P, W2], f32, tag="h2",
                                          name="psum2")
                        nc.tensor.matmul(psum2[:, :4 * P], lhsT=w1_t[:],
                                         rhs=ecat[:, :4 * P],
                                         start=True, stop=True)
                        nc.tensor.matmul(psum2[:, 4 * P:], lhsT=w1_t[:],
                                         rhs=ecat[:, 4 * P:],
                                         start=True, stop=True)
                        if pair_ctr[0] % DIRECT_EVERY == 0:
                            flush_w()
                            nc.vector.reduce_max(
                                out=out_stage[:, t * P:(t + 2) * P],
                                in_=psum2[:].rearrange(
                                    "p (u j q) -> p u q j", u=2, q=P),
                                axis=mybir.AxisListType.X)
                        else:
                            h16 = hp.tile([P, W2], f16, tag="h16b",
                                          name="h16b")
                            nc.scalar.activation(
                                h16[:], psum2[:],
                                mybir.ActivationFunctionType.Identity)
                            w_add(t, h16, 0)
                            w_add(t + 1, h16, 4 * P)
                        t += 2
                        continue

                    # m-max + matmul per pass; psums kept for this tile
                    psums = []
                    for s0 in range(0, G, PASS_PANELS):
                        gp = min(PASS_PANELS, G - s0)
                        w = gp * P
                        ecat = ep.tile([P, PASS_PANELS * P], f16, tag="e")
                        nc.vector.tensor_tensor(
                            out=ecat[:, :w].rearrange("p (j q) -> p j q", q=P),
                            in0=zg_t[:, toff + s0 * P:toff + (s0 + gp) * P]
                                .rearrange("p (j q) -> p j q", q=P),
                            in1=b4_s.rearrange("p (j q) -> p j q", j=1)
                                .to_broadcast([P, gp, P]),
                            op=mybir.AluOpType.max)
                        psum = hps.tile([P, PASS_PANELS * P], f32, tag="h")
                        nc.tensor.matmul(psum[:, :w], lhsT=w1_t[:],
                                         rhs=ecat[:, :w], start=True, stop=True)
                        psums.append((psum, gp))

                    if G == 1:
                        flush_w()
                        nc.scalar.activation(
                            ostage, psums[0][0][:, :P],
                            mybir.ActivationFunctionType.Identity)
                        t += 1
                        continue
                    if G <= 3:
                        # small tile: direct DVE reduce from PSUM
                        flush_w()
                        nc.vector.reduce_max(
                            out=ostage,
                            in_=psums[0][0][:, :G * P]
                                .rearrange("p (j q) -> p q j", q=P),
                            axis=mybir.AxisListType.X)
                        t += 1
                        continue
                    if G == 4:
                        # lone G=4 tile: evict + ttA into batched W
                        h16 = hp.tile([P, PASS_PANELS * P], f16, tag="h16")
                        nc.scalar.activation(
                            h16[:, :4 * P], psums[0][0][:, :4 * P],
                            mybir.ActivationFunctionType.Identity)
                        w_add(t, h16, 0)
                        t += 1
                        continue
                    # multi-pass tile: evict all passes into one h16 staging
                    flush_w()
                    h16 = hp.tile([P, HMAX], f16, tag="hbig")
                    for pi, (psum, gp) in enumerate(psums):
                        nc.scalar.activation(
                            h16[:, pi * 4 * P:pi * 4 * P + gp * P],
                            psum[:, :gp * P],
                            mybir.ActivationFunctionType.Identity)
                    # level-wise wide max tree over the G panels in h16;
                    # odd leftovers are folded in at the end
                    cur, nb = h16, G
                    extras = []
                    done = False
                    while nb > 1:
                        if nb % 2:
                            extras.append(cur[:, (nb - 1) * P:nb * P])
                            nb -= 1
                        half = nb // 2
                        if half == 1 and not extras:
                            tt_max(ostage, cur[:, 0:P], cur[:, P:2 * P])
                            done = True
                            break
                        nxt = pp.tile([P, HMAX // 2], f16, tag="lvl",
                                      name="lvl_t")
                        ev = cur[:, :nb * P].rearrange(
                            "p (k h) -> p k h", h=2 * P)
                        tt_max(nxt[:, :half * P], ev[:, :, 0:P],
                               ev[:, :, P:2 * P])
                        cur, nb = nxt, half
                    if not done:
                        # fold the tree result and extras into out_stage
                        chain = [cur[:, 0:P]] + extras
                        while len(chain) > 2:
                            m_t = pp.tile([P, P], f16, tag="pB",
                                          name="pB_t")
                            tt_max(m_t[:], chain[0], chain[1])
                            chain = [m_t[:]] + chain[2:]
                        tt_max(ostage, chain[0], chain[1])
                    t += 1
                flush_w()
                # stream finished tiles out
                nc.sync.dma_start(out=out_d[:, ta * P:tb * P],
                                  in_=out_stage[:, ta * P:tb * P])
    nc.finalize()
    return nc


def prepare(inputs):
    """Returns (nc, in_maps, postprocess)."""
    t0 = time.time()
    folded = _fold_weights(inputs)
    cores, meta = _plan(inputs)
    TIMES["plan"] = time.time() - t0
    t0 = time.time()
    nc = _build_program(meta)
    TIMES["build_program"] = time.time() - t0
    t0 = time.time()
    in_maps = []
    host = []
    for core in cores:
        zg, b4, Cq = _build_core_arrays(core, meta, inputs, folded)
        in_maps.append({"zg": zg, "b4": b4, "w1bd": folded["w1bd"]})
        host.append(Cq)
    TIMES["core_arrays"] = time.time() - t0

    def post(results):
        qg, n_dummy = meta["qg"], meta["n_dummy"]
        parts = []
        for ci, core in enumerate(cores):
            raw = np.asarray(results[ci]["out"]).astype(np.float32)
            red = raw.reshape(4, 32, meta["qg_pad"]).max(axis=0).T
            val = np.maximum(red - host[ci], 0.0)      # [qg_pad, 32]
            partial = np.zeros((qg, 32), np.float32)
            partial[core["qrow"][n_dummy:]] = val[n_dummy:]
            partial[core["deg"] == 0] = 0.0
            parts.append(partial[:core["nq_local"]])
        combined = [np.maximum(parts[2 * g], parts[2 * g + 1]) for g in range(4)]
        return np.concatenate(combined, axis=0).astype(np.float32)

    return nc, in_maps, post


def kernel(**inputs):
    from concourse.bass_utils import run_bass_kernel_spmd
    nc, in_maps, post = prepare(inputs)
    t0 = time.time()
    res = run_bass_kernel_spmd(nc, in_maps, core_ids=list(range(8)))
    TIMES["run"] = time.time() - t0
    print("kernel timings:", {k: round(v, 2) for k, v in TIMES.items()},
          flush=True)
    return post(res.results)


# revision 27
# speedup vs baseline: 1.1431x; 1.1431x over previous
"""EdgeConv message-passing kernel for 8 Trainium2 NeuronCores.

Strategy (host-materialized edge tensor + dense streaming device kernel):
  - Queries are range-partitioned into 4 groups; refs into 2 halves. Core c
    handles query group c>>1 and ref half c&1 (its edges are those with
    e_query in the group and e_ref in the half).
  - All BatchNorms fold at inference:
        pre-relu edge feature  z_e = Z[e_ref] - B[e_query]
        Z[n] = ref_bxyz[n,1:4] @ Wp' + ref_feat[n] @ Wf' + (t0+tf)  [N_ref,32]
        B[q] = q_bxyz[q,1:4] @ Wp'                                  [N_q, 32]
        h_e  = relu(z_e) @ W1' + b1',   out[q] = relu(max_e h_e)  (empty -> 0)
    Using relu(Z-B) = max(Z,B) - B and max_e (x_e - c_q) = (max_e x_e) - c_q:
        out[q] = relu( max_e (max(Z[e_ref],B[q]) @ W1') - C'[q] ),
        C'[q] = B[q] @ W1' - b1'.
  - The host materializes the gather Z[e_ref] per edge slot (pure input
    permutation, like the baseline's packing) in fp16, already laid out as
    the device wants it: queries degree-sorted into tiles of 128; slots
    padded to a multiple of 4 (repeating real edges - idempotent under max);
    groups of 4 slot-blocks stacked on the 128 SBUF partitions (panels).
  - The device streams the edge tensor with dense HWDGE DMA and runs, per
    128-query tile: a DVE max against the resident B table (fp16 2x mode),
    one block-diagonal W1' matmul per <=4 panels (fp16), an ACT-engine
    PSUM -> fp16 SBUF copy, and a DVE tensor-tensor max tree over panels.
    Output [128, qg_pad] fp16 is DMA'd out in chunks as tiles finish.
  - Host post: max over the 4 partition blocks, relu(red - C'), empty-query
    zeroing, inverse permutation, and pairwise max of the two ref halves.
"""
import time

import numpy as np

import concourse.bass as bass
import concourse.tile as tile
from concourse import bacc, mybir

EPS = 1e-3
P = 128
DMA_COLS = 8192      # max zg columns per streamed DMA group
PASS_BLOCKS = 4      # slot-blocks stacked per panel (partition blocks)
PASS_PANELS = 4      # panels per matmul pass (<= 512 psum cols)

TIMES = {}


def _fold_weights(inputs):
    f = np.float32
    s0 = inputs["bn0_g"] / np.sqrt(inputs["bn0_v"] + EPS)
    t0 = inputs["bn0_b"] - inputs["bn0_m"] * s0
    sf = inputs["bnf_g"] / np.sqrt(inputs["bnf_v"] + EPS)
    tf = inputs["bnf_b"] - inputs["bnf_m"] * sf
    s1 = inputs["bn1_g"] / np.sqrt(inputs["bn1_v"] + EPS)
    t1 = inputs["bn1_b"] - inputs["bn1_m"] * s1

    Wp = (np.asarray(inputs["w_pos"]) * s0).astype(f)      # [3, 32]
    Wf = (np.asarray(inputs["w_feat"]) * sf).astype(f)     # [16, 32]
    cz = (t0 + tf).astype(f)                               # [32]
    W1 = (np.asarray(inputs["w1"]) * s1).astype(np.float16)
    c1 = (np.asarray(inputs["b1"]) * s1 + t1).astype(f)    # [32]

    w1bd = np.zeros((P, P), np.float16)
    for b in range(4):
        w1bd[b * 32:(b + 1) * 32, b * 32:(b + 1) * 32] = W1

    Z = (np.asarray(inputs["ref_bxyz"])[:, 1:4] @ Wp
         + np.asarray(inputs["ref_feat"]) @ Wf + cz).astype(np.float16)
    return {"Wp": Wp, "W1": W1, "c1": c1, "w1bd": w1bd, "Z": Z}


def _plan(inputs):
    """Host-side partitioning: per-core tile schedules (int bookkeeping)."""
    e_ref = np.asarray(inputs["e_ref"]).astype(np.int64)
    e_query = np.asarray(inputs["e_query"]).astype(np.int64)
    n_ref = inputs["ref_bxyz"].shape[0]
    n_q = inputs["query_bxyz"].shape[0]
    half = (n_ref + 1) // 2
    qg = (n_q + 3) // 4
    qg_pad = ((qg + P - 1) // P) * P
    n_tiles = qg_pad // P
    n_dummy = qg_pad - qg

    cores = []
    for g in range(4):
        qlo, qhi = g * qg, min((g + 1) * qg, n_q)
        for h in range(2):
            m = (e_query >= qlo) & (e_query < qhi) & \
                (e_ref >= h * half) & (e_ref < min((h + 1) * half, n_ref))
            er = (e_ref[m] - h * half).astype(np.int64)
            eq = (e_query[m] - qlo).astype(np.int64)
            deg = np.bincount(eq, minlength=qg)
            order = np.argsort(eq, kind="stable")
            er_s = er[order]
            ptr = np.zeros(qg + 1, np.int64)
            np.cumsum(deg, out=ptr[1:])
            perm = np.argsort(deg, kind="stable")      # ascending degree
            qrow = np.full(qg_pad, -1, np.int64)
            qrow[n_dummy:] = perm
            degrow = np.zeros(qg_pad, np.int64)
            degrow[n_dummy:] = deg[perm]
            ptrrow = np.zeros(qg_pad, np.int64)
            ptrrow[n_dummy:] = ptr[perm]
            kt = degrow.reshape(n_tiles, P).max(axis=1)
            kt = np.maximum(kt, 1)
            cores.append({
                "g": g, "h": h, "qlo": qlo, "nq_local": qhi - qlo,
                "er_s": er_s, "deg": deg, "qrow": qrow,
                "degrow": degrow, "ptrrow": ptrrow, "kt": kt,
            })

    # shared slot schedule across the 8 SPMD cores; blocks padded to mult of 4
    kmax = np.max(np.stack([c["kt"] for c in cores]), axis=0)
    k4 = ((kmax + PASS_BLOCKS - 1) // PASS_BLOCKS) * PASS_BLOCKS
    panels = k4 // PASS_BLOCKS                     # panels per tile
    cols = panels * P                              # zg columns per tile
    col_off = np.zeros(n_tiles + 1, np.int64)
    np.cumsum(cols, out=col_off[1:])

    # dma groups: consecutive tiles, <= DMA_COLS columns each
    groups = []
    t = 0
    while t < n_tiles:
        t0_, n = t, 0
        while t < n_tiles and n + int(cols[t]) <= DMA_COLS:
            n += int(cols[t])
            t += 1
        assert t > t0_, f"tile {t0_} alone exceeds DMA_COLS"
        groups.append((t0_, t, n))

    meta = {
        "half": half, "qg": qg, "qg_pad": qg_pad, "n_tiles": n_tiles,
        "n_dummy": n_dummy, "panels": panels, "col_off": col_off,
        "groups": groups, "n_q": n_q, "n_ref": n_ref,
        "totcol": int(col_off[-1]),
    }
    return cores, meta


def _build_core_arrays(core, meta, inputs, folded):
    """zg [128, totcol] f16, b4 [128, qg_pad] f16, Cq [qg_pad, 32] f32."""
    half, qg_pad, n_tiles = meta["half"], meta["qg_pad"], meta["n_tiles"]
    panels, col_off = meta["panels"], meta["col_off"]
    n_ref = meta["n_ref"]
    er_s, degrow, ptrrow, qrow = (core["er_s"], core["degrow"],
                                  core["ptrrow"], core["qrow"])

    lo = core["h"] * half
    hi = min(lo + half, n_ref)
    Zl = np.zeros((hi - lo + 1, 32), np.float16)   # +1: safe row for deg 0
    Zl[:hi - lo] = folded["Z"][lo:hi]

    # flat [4, totcol] table-row index: tile t, panel j, block i=partition
    # block, query p  ->  column col_off[t] + j*128 + p, row idx of slot
    # (t, c=4j+i, p)
    idx4 = np.empty((4, meta["totcol"]), np.int64)
    for t in range(n_tiles):
        rows = slice(t * P, (t + 1) * P)
        K4 = int(panels[t]) * PASS_BLOCKS
        d = np.maximum(degrow[rows], 1)[:, None]
        j = np.arange(K4)[None, :]
        pos = ptrrow[rows][:, None] + (j % d)          # [128, K4]
        if er_s.size:
            it = er_s[np.minimum(pos, er_s.size - 1)]
        else:
            it = np.zeros((P, K4), np.int64)
        it = np.where(degrow[rows][:, None] > 0, it, hi - lo)
        # [128 p, K4] -> [G, 4, 128] -> [4, G*128]
        itt = it.T.reshape(K4 // 4, 4, P).transpose(1, 0, 2).reshape(4, -1)
        idx4[:, col_off[t]:col_off[t + 1]] = itt
    # one big gather + partition-major layout
    zg = np.ascontiguousarray(
        Zl[idx4].transpose(0, 2, 1).reshape(P, meta["totcol"]))

    qx = np.zeros((qg_pad, 3), np.float32)
    valid = qrow >= 0
    qx[valid] = np.asarray(inputs["query_bxyz"])[core["qlo"] + qrow[valid], 1:4]
    B = (qx @ folded["Wp"]).astype(np.float16)         # [qg_pad, 32]
    b4 = np.ascontiguousarray(np.tile(B.T, (4, 1)))    # [128, qg_pad] f16
    Cq = (B.astype(np.float32) @ folded["W1"].astype(np.float32)
          - folded["c1"]).astype(np.float32)           # [qg_pad, 32]
    return zg, b4, Cq


def _build_program(meta):
    f16 = mybir.dt.float16
    f32 = mybir.dt.float32
    qg_pad, n_tiles = meta["qg_pad"], meta["n_tiles"]
    panels, col_off, groups = meta["panels"], meta["col_off"], meta["groups"]

    nc = bacc.Bacc("TRN2", num_devices=8)
    zg_d = nc.dram_tensor("zg", [P, meta["totcol"]], f16, kind="ExternalInput")
    b4_d = nc.dram_tensor("b4", [P, qg_pad], f16, kind="ExternalInput")
    w1_d = nc.dram_tensor("w1bd", [P, P], f16, kind="ExternalInput")
    out_d = nc.dram_tensor("out", [P, qg_pad], f16, kind="ExternalOutput")

    with tile.TileContext(nc) as tc:
        with tc.tile_pool(name="const", bufs=1) as cp, \
             tc.tile_pool(name="bpool", bufs=3) as bp, \
             tc.tile_pool(name="zpool", bufs=3) as zp, \
             tc.tile_pool(name="epool", bufs=3) as ep, \
             tc.tile_pool(name="hpool", bufs=4) as hp, \
             tc.tile_pool(name="ppool", bufs=8) as pp, \
             tc.tile_pool(name="hps", bufs=6, space="PSUM") as hps:
            w1_t = cp.tile([P, P], f16, name="w1_t")
            nc.sync.dma_start(out=w1_t[:], in_=w1_d[:])
            out_stage = cp.tile([P, qg_pad], f16, name="out_stage")

            def tt_max(dst, a, b):
                nc.vector.tensor_tensor(out=dst, in0=a, in1=b,
                                        op=mybir.AluOpType.max)

            HMAX = 10 * P        # per-tile h16 staging width (max panels)

            # pending G=4 tiles whose ttA halves sit in the current W tile
            wstate = {"tile": None, "n": 0, "t0": -1}

            def flush_w():
                n, t0 = wstate["n"], wstate["t0"]
                if wstate["tile"] is None or n == 0:
                    return
                wt = wstate["tile"]
                dst = out_stage[:, t0 * P:(t0 + n) * P]
                if n == 1:
                    tt_max(dst, wt[:, 0:P], wt[:, P:2 * P])
                else:
                    tt_max(dst,
                           wt[:].rearrange("p (k h) -> p k h", h=2 * P)
                               [:, :n, 0:P],
                           wt[:].rearrange("p (k h) -> p k h", h=2 * P)
                               [:, :n, P:2 * P])
                wstate["tile"] = None
                wstate["n"] = 0

            def w_add(t, h16, off):
                """ttA for a G=4 tile's 4 panels (at h16 col off) into W."""
                if wstate["tile"] is not None and \
                        wstate["t0"] + wstate["n"] != t:
                    flush_w()
                if wstate["tile"] is None:
                    wstate["tile"] = pp.tile([P, 4 * 2 * P], f16,
                                             tag="W", name="w_t")
                    wstate["t0"] = t
                    wstate["n"] = 0
                k = wstate["n"]
                tt_max(wstate["tile"][:, k * 2 * P:(k + 1) * 2 * P],
                       h16[:, off:off + 2 * P],
                       h16[:, off + 2 * P:off + 4 * P])
                wstate["n"] += 1
                if wstate["n"] == 4:
                    flush_w()

            for (ta, tb, ncols) in groups:
                zg_t = zp.tile([P, DMA_COLS], f16, tag="zg")
                base = int(col_off[ta])
                nc.sync.dma_start(out=zg_t[:, :ncols],
                                  in_=zg_d[:, base:base + ncols])
                b4_t = bp.tile([P, DMA_COLS], f16, tag="b4")
                nc.sync.dma_start(out=b4_t[:, :(tb - ta) * P],
                                  in_=b4_d[:, ta * P:tb * P])
                t = ta
                while t < tb:
                    G = int(panels[t])
                    toff = int(col_off[t]) - base
                    b4_s = b4_t[:, (t - ta) * P:(t - ta + 1) * P]
                    ostage = out_stage[:, t * P:(t + 1) * P]


                    # m-max + matmul per pass; psums kept for this tile
                    psums = []
                    for s0 in range(0, G, PASS_PANELS):
                        gp = min(PASS_PANELS, G - s0)
                        w = gp * P
                        ecat = ep.tile([P, PASS_PANELS * P], f16, tag="e")
                        nc.vector.tensor_tensor(
                            out=ecat[:, :w].rearrange("p (j q) -> p j q", q=P),
                            in0=zg_t[:, toff + s0 * P:toff + (s0 + gp) * P]
                                .rearrange("p (j q) -> p j q", q=P),
                            in1=b4_s.rearrange("p (j q) -> p j q", j=1)
                                .to_broadcast([P, gp, P]),
                            op=mybir.AluOpType.max)
                        psum = hps.tile([P, PASS_PANELS * P], f32, tag="h")
                        nc.tensor.matmul(psum[:, :w], lhsT=w1_t[:],
                                         rhs=ecat[:, :w], start=True, stop=True)
                        psums.append((psum, gp))

                    if G == 1:
                        flush_w()
                        nc.scalar.activation(
                            ostage, psums[0][0][:, :P],
                            mybir.ActivationFunctionType.Identity)
                        t += 1
                        continue
                    if G <= 3:
                        # small tile: direct DVE reduce from PSUM
                        flush_w()
                        nc.vector.reduce_max(
                            out=ostage,
                            in_=psums[0][0][:, :G * P]
                                .rearrange("p (j q) -> p q j", q=P),
                            axis=mybir.AxisListType.X)
                        t += 1
                        continue
                    if G == 4:
                        # lone G=4 tile: evict + ttA into batched W
                        h16 = hp.tile([P, PASS_PANELS * P], f16, tag="h16")
                        nc.scalar.activation(
                            h16[:, :4 * P], psums[0][0][:, :4 * P],
                            mybir.ActivationFunctionType.Identity)
                        w_add(t, h16, 0)
                        t += 1
                        continue
                    # multi-pass tile: evict all passes into one h16 staging
                    flush_w()
                    h16 = hp.tile([P, HMAX], f16, tag="hbig")
                    for pi, (psum, gp) in enumerate(psums):
                        nc.scalar.activation(
                            h16[:, pi * 4 * P:pi * 4 * P + gp * P],
                            psum[:, :gp * P],
                            mybir.ActivationFunctionType.Identity)
                    # level-wise wide max tree over the G panels in h16;
                    # odd leftovers are folded in at the end
                    cur, nb = h16, G
                    extras = []
                    done = False
                    while nb > 1:
                        if nb % 2:
                            extras.append(cur[:, (nb - 1) * P:nb * P])
                            nb -= 1
                        half = nb // 2
                        if half == 1 and not extras:
                            tt_max(ostage, cur[:, 0:P], cur[:, P:2 * P])
                            done = True
                            break
                        nxt = pp.tile([P, HMAX // 2], f16, tag="lvl",
                                      name="lvl_t")
                        ev = cur[:, :nb * P].rearrange(
                            "p (k h) -> p k h", h=2 * P)
                        tt_max(nxt[:, :half * P], ev[:, :, 0:P],
                               ev[:, :, P:2 * P])
                        cur, nb = nxt, half
                    if not done:
                        # fold the tree result and extras into out_stage
                        chain = [cur[:, 0:P]] + extras
                        while len(chain) > 2:
                            m_t = pp.tile([P, P], f16, tag="pB",
                                          name="pB_t")
                            tt_max(m_t[:], chain[0], chain[1])
                            chain = [m_t[:]] + chain[2:]
                        tt_max(ostage, chain[0], chain[1])
                    t += 1
                flush_w()
                # stream finished tiles out
                nc.sync.dma_start(out=out_d[:, ta * P:tb * P],
                                  in_=out_stage[:, ta * P:tb * P])
    nc.finalize()
    return nc


def prepare(inputs):
    """Returns (nc, in_maps, postprocess)."""
    t0 = time.time()
    folded = _fold_weights(inputs)
    cores, meta = _plan(inputs)
    TIMES["plan"] = time.time() - t0
    t0 = time.time()
    nc = _build_program(meta)
    TIMES["build_program"] = time.time() - t0
    t0 = time.time()
    in_maps = []
    host = []
    for core in cores:
        zg, b4, Cq = _build_core_arrays(core, meta, inputs, folded)
        in_maps.append({"zg": zg, "b4": b4, "w1bd": folded["w1bd"]})
        host.append(Cq)
    TIMES["core_arrays"] = time.time() - t0

    def post(results):
        qg, n_dummy = meta["qg"], meta["n_dummy"]
        parts = []
        for ci, core in enumerate(cores):
            raw = np.asarray(results[ci]["out"]).astype(np.float32)
            red = raw.reshape(4, 32, meta["qg_pad"]).max(axis=0).T
            val = np.maximum(red - host[ci], 0.0)      # [qg_pad, 32]
            partial = np.zeros((qg, 32), np.float32)
            partial[core["qrow"][n_dummy:]] = val[n_dummy:]
            partial[core["deg"] == 0] = 0.0
            parts.append(partial[:core["nq_local"]])
        combined = [np.maximum(parts[2 * g], parts[2 * g + 1]) for g in range(4)]
        return np.concatenate(combined, axis=0).astype(np.float32)

    return nc, in_maps, post


def kernel(**inputs):
    from concourse.bass_utils import run_bass_kernel_spmd
    nc, in_maps, post = prepare(inputs)
    t0 = time.time()
    res = run_bass_kernel_spmd(nc, in_maps, core_ids=list(range(8)))
    TIMES["run"] = time.time() - t0
    return post(res.results)


# revision 28
# speedup vs baseline: 1.1491x; 1.0053x over previous
"""EdgeConv message-passing kernel for 8 Trainium2 NeuronCores.

Strategy (host-materialized edge tensor + dense streaming device kernel):
  - Queries are range-partitioned into 4 groups; refs into 2 halves. Core c
    handles query group c>>1 and ref half c&1 (its edges are those with
    e_query in the group and e_ref in the half).
  - All BatchNorms fold at inference:
        pre-relu edge feature  z_e = Z[e_ref] - B[e_query]
        Z[n] = ref_bxyz[n,1:4] @ Wp' + ref_feat[n] @ Wf' + (t0+tf)  [N_ref,32]
        B[q] = q_bxyz[q,1:4] @ Wp'                                  [N_q, 32]
        h_e  = relu(z_e) @ W1' + b1',   out[q] = relu(max_e h_e)  (empty -> 0)
    Using relu(Z-B) = max(Z,B) - B and max_e (x_e - c_q) = (max_e x_e) - c_q:
        out[q] = relu( max_e (max(Z[e_ref],B[q]) @ W1') - C'[q] ),
        C'[q] = B[q] @ W1' - b1'.
  - The host materializes the gather Z[e_ref] per edge slot (pure input
    permutation, like the baseline's packing) in fp16, already laid out as
    the device wants it: queries degree-sorted into tiles of 128; slots
    padded to a multiple of 4 (repeating real edges - idempotent under max);
    groups of 4 slot-blocks stacked on the 128 SBUF partitions (panels).
  - The device streams the edge tensor with dense HWDGE DMA and runs, per
    128-query tile: a DVE max against the resident B table (fp16 2x mode),
    one block-diagonal W1' matmul per <=4 panels (fp16), an ACT-engine
    PSUM -> fp16 SBUF copy, and a DVE tensor-tensor max tree over panels.
    Output [128, qg_pad] fp16 is DMA'd out in chunks as tiles finish.
  - Host post: max over the 4 partition blocks, relu(red - C'), empty-query
    zeroing, inverse permutation, and pairwise max of the two ref halves.
"""
import time

import numpy as np

import concourse.bass as bass
import concourse.tile as tile
from concourse import bacc, mybir

EPS = 1e-3
P = 128
DMA_COLS = 8192      # max zg columns per streamed DMA group
PASS_BLOCKS = 4      # slot-blocks stacked per panel (partition blocks)
PASS_PANELS = 4      # panels per matmul pass (<= 512 psum cols)

TIMES = {}


def _fold_weights(inputs):
    f = np.float32
    s0 = inputs["bn0_g"] / np.sqrt(inputs["bn0_v"] + EPS)
    t0 = inputs["bn0_b"] - inputs["bn0_m"] * s0
    sf = inputs["bnf_g"] / np.sqrt(inputs["bnf_v"] + EPS)
    tf = inputs["bnf_b"] - inputs["bnf_m"] * sf
    s1 = inputs["bn1_g"] / np.sqrt(inputs["bn1_v"] + EPS)
    t1 = inputs["bn1_b"] - inputs["bn1_m"] * s1

    Wp = (np.asarray(inputs["w_pos"]) * s0).astype(f)      # [3, 32]
    Wf = (np.asarray(inputs["w_feat"]) * sf).astype(f)     # [16, 32]
    cz = (t0 + tf).astype(f)                               # [32]
    W1 = (np.asarray(inputs["w1"]) * s1).astype(np.float16)
    c1 = (np.asarray(inputs["b1"]) * s1 + t1).astype(f)    # [32]

    w1bd = np.zeros((P, P), np.float16)
    for b in range(4):
        w1bd[b * 32:(b + 1) * 32, b * 32:(b + 1) * 32] = W1

    Z = (np.asarray(inputs["ref_bxyz"])[:, 1:4] @ Wp
         + np.asarray(inputs["ref_feat"]) @ Wf + cz).astype(np.float16)
    return {"Wp": Wp, "W1": W1, "c1": c1, "w1bd": w1bd, "Z": Z}


def _plan(inputs):
    """Host-side partitioning: per-core tile schedules (int bookkeeping)."""
    e_ref = np.asarray(inputs["e_ref"]).astype(np.int64)
    e_query = np.asarray(inputs["e_query"]).astype(np.int64)
    n_ref = inputs["ref_bxyz"].shape[0]
    n_q = inputs["query_bxyz"].shape[0]
    half = (n_ref + 1) // 2
    qg = (n_q + 3) // 4
    qg_pad = ((qg + P - 1) // P) * P
    n_tiles = qg_pad // P
    n_dummy = qg_pad - qg

    cores = []
    for g in range(4):
        qlo, qhi = g * qg, min((g + 1) * qg, n_q)
        for h in range(2):
            m = (e_query >= qlo) & (e_query < qhi) & \
                (e_ref >= h * half) & (e_ref < min((h + 1) * half, n_ref))
            er = (e_ref[m] - h * half).astype(np.int64)
            eq = (e_query[m] - qlo).astype(np.int64)
            deg = np.bincount(eq, minlength=qg)
            order = np.argsort(eq, kind="stable")
            er_s = er[order]
            ptr = np.zeros(qg + 1, np.int64)
            np.cumsum(deg, out=ptr[1:])
            perm = np.argsort(deg, kind="stable")      # ascending degree
            qrow = np.full(qg_pad, -1, np.int64)
            qrow[n_dummy:] = perm
            degrow = np.zeros(qg_pad, np.int64)
            degrow[n_dummy:] = deg[perm]
            ptrrow = np.zeros(qg_pad, np.int64)
            ptrrow[n_dummy:] = ptr[perm]
            kt = degrow.reshape(n_tiles, P).max(axis=1)
            kt = np.maximum(kt, 1)
            cores.append({
                "g": g, "h": h, "qlo": qlo, "nq_local": qhi - qlo,
                "er_s": er_s, "deg": deg, "qrow": qrow,
                "degrow": degrow, "ptrrow": ptrrow, "kt": kt,
            })

    # shared slot schedule across the 8 SPMD cores; blocks padded to mult of 4
    kmax = np.max(np.stack([c["kt"] for c in cores]), axis=0)
    k4 = ((kmax + PASS_BLOCKS - 1) // PASS_BLOCKS) * PASS_BLOCKS
    panels = k4 // PASS_BLOCKS                     # panels per tile
    cols = panels * P                              # zg columns per tile
    col_off = np.zeros(n_tiles + 1, np.int64)
    np.cumsum(cols, out=col_off[1:])

    # dma groups: consecutive tiles, <= DMA_COLS columns each
    groups = []
    t = 0
    while t < n_tiles:
        t0_, n = t, 0
        while t < n_tiles and n + int(cols[t]) <= DMA_COLS:
            n += int(cols[t])
            t += 1
        assert t > t0_, f"tile {t0_} alone exceeds DMA_COLS"
        groups.append((t0_, t, n))

    meta = {
        "half": half, "qg": qg, "qg_pad": qg_pad, "n_tiles": n_tiles,
        "n_dummy": n_dummy, "panels": panels, "col_off": col_off,
        "groups": groups, "n_q": n_q, "n_ref": n_ref,
        "totcol": int(col_off[-1]),
    }
    return cores, meta


def _build_core_arrays(core, meta, inputs, folded):
    """zg [128, totcol] f16, b4 [128, qg_pad] f16, Cq [qg_pad, 32] f32."""
    half, qg_pad, n_tiles = meta["half"], meta["qg_pad"], meta["n_tiles"]
    panels, col_off = meta["panels"], meta["col_off"]
    n_ref = meta["n_ref"]
    er_s, degrow, ptrrow, qrow = (core["er_s"], core["degrow"],
                                  core["ptrrow"], core["qrow"])

    lo = core["h"] * half
    hi = min(lo + half, n_ref)
    Zl = np.zeros((hi - lo + 1, 32), np.float16)   # +1: safe row for deg 0
    Zl[:hi - lo] = folded["Z"][lo:hi]

    # flat [4, totcol] table-row index: tile t, panel j, block i=partition
    # block, query p  ->  column col_off[t] + j*128 + p, row idx of slot
    # (t, c=4j+i, p)
    idx4 = np.empty((4, meta["totcol"]), np.int64)
    for t in range(n_tiles):
        rows = slice(t * P, (t + 1) * P)
        K4 = int(panels[t]) * PASS_BLOCKS
        d = np.maximum(degrow[rows], 1)[:, None]
        j = np.arange(K4)[None, :]
        pos = ptrrow[rows][:, None] + (j % d)          # [128, K4]
        if er_s.size:
            it = er_s[np.minimum(pos, er_s.size - 1)]
        else:
            it = np.zeros((P, K4), np.int64)
        it = np.where(degrow[rows][:, None] > 0, it, hi - lo)
        # [128 p, K4] -> [G, 4, 128] -> [4, G*128]
        itt = it.T.reshape(K4 // 4, 4, P).transpose(1, 0, 2).reshape(4, -1)
        idx4[:, col_off[t]:col_off[t + 1]] = itt
    # one big gather + partition-major layout
    zg = np.ascontiguousarray(
        Zl[idx4].transpose(0, 2, 1).reshape(P, meta["totcol"]))

    qx = np.zeros((qg_pad, 3), np.float32)
    valid = qrow >= 0
    qx[valid] = np.asarray(inputs["query_bxyz"])[core["qlo"] + qrow[valid], 1:4]
    B = (qx @ folded["Wp"]).astype(np.float16)         # [qg_pad, 32]
    b4 = np.ascontiguousarray(np.tile(B.T, (4, 1)))    # [128, qg_pad] f16
    Cq = (B.astype(np.float32) @ folded["W1"].astype(np.float32)
          - folded["c1"]).astype(np.float32)           # [qg_pad, 32]
    return zg, b4, Cq


def _build_program(meta):
    f16 = mybir.dt.float16
    f32 = mybir.dt.float32
    qg_pad, n_tiles = meta["qg_pad"], meta["n_tiles"]
    panels, col_off, groups = meta["panels"], meta["col_off"], meta["groups"]

    nc = bacc.Bacc("TRN2", num_devices=8)
    zg_d = nc.dram_tensor("zg", [P, meta["totcol"]], f16, kind="ExternalInput")
    b4_d = nc.dram_tensor("b4", [P, qg_pad], f16, kind="ExternalInput")
    w1_d = nc.dram_tensor("w1bd", [P, P], f16, kind="ExternalInput")
    out_d = nc.dram_tensor("out", [P, qg_pad], f16, kind="ExternalOutput")

    with tile.TileContext(nc) as tc:
        with tc.tile_pool(name="const", bufs=1) as cp, \
             tc.tile_pool(name="bpool", bufs=3) as bp, \
             tc.tile_pool(name="zpool", bufs=3) as zp, \
             tc.tile_pool(name="epool", bufs=3) as ep, \
             tc.tile_pool(name="hpool", bufs=4) as hp, \
             tc.tile_pool(name="ppool", bufs=8) as pp, \
             tc.tile_pool(name="hps", bufs=6, space="PSUM") as hps:
            w1_t = cp.tile([P, P], f16, name="w1_t")
            nc.sync.dma_start(out=w1_t[:], in_=w1_d[:])
            out_stage = cp.tile([P, qg_pad], f16, name="out_stage")

            def tt_max(dst, a, b):
                nc.vector.tensor_tensor(out=dst, in0=a, in1=b,
                                        op=mybir.AluOpType.max)

            HMAX = 10 * P        # per-tile h16 staging width (max panels)

            # pending G=4 tiles whose ttA halves sit in the current W tile
            wstate = {"tile": None, "n": 0, "t0": -1}

            def flush_w():
                n, t0 = wstate["n"], wstate["t0"]
                if wstate["tile"] is None or n == 0:
                    return
                wt = wstate["tile"]
                dst = out_stage[:, t0 * P:(t0 + n) * P]
                tt_max(dst, wt[:, :n * P], wt[:, 4 * P:(4 + n) * P])
                wstate["tile"] = None
                wstate["n"] = 0

            def w_add(t, h16, off):
                """ttA for a G=4 tile's 4 panels (at h16 col off) into W."""
                if wstate["tile"] is not None and \
                        wstate["t0"] + wstate["n"] != t:
                    flush_w()
                if wstate["tile"] is None:
                    wstate["tile"] = pp.tile([P, 4 * 2 * P], f16,
                                             tag="W", name="w_t")
                    wstate["t0"] = t
                    wstate["n"] = 0
                k = wstate["n"]
                wv = wstate["tile"][:].rearrange("p (h z) -> p h z", h=2)
                tt_max(wv[:, :, k * P:(k + 1) * P],
                       h16[:, off:off + 2 * P],
                       h16[:, off + 2 * P:off + 4 * P])
                wstate["n"] += 1
                if wstate["n"] == 4:
                    flush_w()

            for (ta, tb, ncols) in groups:
                zg_t = zp.tile([P, DMA_COLS], f16, tag="zg")
                base = int(col_off[ta])
                nc.sync.dma_start(out=zg_t[:, :ncols],
                                  in_=zg_d[:, base:base + ncols])
                b4_t = bp.tile([P, DMA_COLS], f16, tag="b4")
                nc.sync.dma_start(out=b4_t[:, :(tb - ta) * P],
                                  in_=b4_d[:, ta * P:tb * P])
                t = ta
                while t < tb:
                    G = int(panels[t])
                    toff = int(col_off[t]) - base
                    b4_s = b4_t[:, (t - ta) * P:(t - ta + 1) * P]
                    ostage = out_stage[:, t * P:(t + 1) * P]


                    # m-max + matmul per pass; psums kept for this tile
                    psums = []
                    for s0 in range(0, G, PASS_PANELS):
                        gp = min(PASS_PANELS, G - s0)
                        w = gp * P
                        ecat = ep.tile([P, PASS_PANELS * P], f16, tag="e")
                        nc.vector.tensor_tensor(
                            out=ecat[:, :w].rearrange("p (j q) -> p j q", q=P),
                            in0=zg_t[:, toff + s0 * P:toff + (s0 + gp) * P]
                                .rearrange("p (j q) -> p j q", q=P),
                            in1=b4_s.rearrange("p (j q) -> p j q", j=1)
                                .to_broadcast([P, gp, P]),
                            op=mybir.AluOpType.max)
                        psum = hps.tile([P, PASS_PANELS * P], f32, tag="h")
                        nc.tensor.matmul(psum[:, :w], lhsT=w1_t[:],
                                         rhs=ecat[:, :w], start=True, stop=True)
                        psums.append((psum, gp))

                    if G == 1:
                        flush_w()
                        nc.scalar.activation(
                            ostage, psums[0][0][:, :P],
                            mybir.ActivationFunctionType.Identity)
                        t += 1
                        continue
                    if G <= 3:
                        # small tile: direct DVE reduce from PSUM
                        flush_w()
                        nc.vector.reduce_max(
                            out=ostage,
                            in_=psums[0][0][:, :G * P]
                                .rearrange("p (j q) -> p q j", q=P),
                            axis=mybir.AxisListType.X)
                        t += 1
                        continue
                    if G == 4:
                        # lone G=4 tile: evict + ttA into batched W
                        h16 = hp.tile([P, PASS_PANELS * P], f16, tag="h16")
                        nc.scalar.activation(
                            h16[:, :4 * P], psums[0][0][:, :4 * P],
                            mybir.ActivationFunctionType.Identity)
                        w_add(t, h16, 0)
                        t += 1
                        continue
                    # multi-pass tile: evict all passes into one h16 staging
                    flush_w()
                    h16 = hp.tile([P, HMAX], f16, tag="hbig")
                    for pi, (psum, gp) in enumerate(psums):
                        nc.scalar.activation(
                            h16[:, pi * 4 * P:pi * 4 * P + gp * P],
                            psum[:, :gp * P],
                            mybir.ActivationFunctionType.Identity)
                    # level-wise wide max tree over the G panels in h16;
                    # odd leftovers are folded in at the end
                    cur, nb = h16, G
                    extras = []
                    done = False
                    while nb > 1:
                        if nb % 2:
                            extras.append(cur[:, (nb - 1) * P:nb * P])
                            nb -= 1
                        half = nb // 2
                        if half == 1 and not extras:
                            tt_max(ostage, cur[:, 0:P], cur[:, P:2 * P])
                            done = True
                            break
                        nxt = pp.tile([P, HMAX // 2], f16, tag="lvl",
                                      name="lvl_t")
                        tt_max(nxt[:, :half * P], cur[:, :half * P],
                               cur[:, half * P:2 * half * P])
                        cur, nb = nxt, half
                    if not done:
                        # fold the tree result and extras into out_stage
                        chain = [cur[:, 0:P]] + extras
                        while len(chain) > 2:
                            m_t = pp.tile([P, P], f16, tag="pB",
                                          name="pB_t")
                            tt_max(m_t[:], chain[0], chain[1])
                            chain = [m_t[:]] + chain[2:]
                        tt_max(ostage, chain[0], chain[1])
                    t += 1
                flush_w()
                # stream finished tiles out
                nc.sync.dma_start(out=out_d[:, ta * P:tb * P],
                                  in_=out_stage[:, ta * P:tb * P])
    nc.finalize()
    return nc


def prepare(inputs):
    """Returns (nc, in_maps, postprocess)."""
    t0 = time.time()
    folded = _fold_weights(inputs)
    cores, meta = _plan(inputs)
    TIMES["plan"] = time.time() - t0
    t0 = time.time()
    nc = _build_program(meta)
    TIMES["build_program"] = time.time() - t0
    t0 = time.time()
    in_maps = []
    host = []
    for core in cores:
        zg, b4, Cq = _build_core_arrays(core, meta, inputs, folded)
        in_maps.append({"zg": zg, "b4": b4, "w1bd": folded["w1bd"]})
        host.append(Cq)
    TIMES["core_arrays"] = time.time() - t0

    def post(results):
        qg, n_dummy = meta["qg"], meta["n_dummy"]
        parts = []
        for ci, core in enumerate(cores):
            raw = np.asarray(results[ci]["out"]).astype(np.float32)
            red = raw.reshape(4, 32, meta["qg_pad"]).max(axis=0).T
            val = np.maximum(red - host[ci], 0.0)      # [qg_pad, 32]
            partial = np.zeros((qg, 32), np.float32)
            partial[core["qrow"][n_dummy:]] = val[n_dummy:]
            partial[core["deg"] == 0] = 0.0
            parts.append(partial[:core["nq_local"]])
        combined = [np.maximum(parts[2 * g], parts[2 * g + 1]) for g in range(4)]
        return np.concatenate(combined, axis=0).astype(np.float32)

    return nc, in_maps, post


def kernel(**inputs):
    from concourse.bass_utils import run_bass_kernel_spmd
    nc, in_maps, post = prepare(inputs)
    t0 = time.time()
    res = run_bass_kernel_spmd(nc, in_maps, core_ids=list(range(8)))
    TIMES["run"] = time.time() - t0
    return post(res.results)
